# revision 17
# baseline (speedup 1.0000x reference)
"""FP4 block-quantized linear: y = x @ dequant(packed, scales, zeros).T + bias.

Tensor-parallel over out_features across 8 NeuronCores (1536 rows each).

Per-core device pipeline:
  - DVE: batched hi/lo nibble extracts of the packed int32 bytes (bitwise ops
    cannot cast, so int32->int32), one pair per half row-block.
  - DVE: per 128x128 tile, one arith op converts int32->fp16 and adds the
    per-partition zos = zeros/scales, producing qn[o, i'] where
    i' = (even i | odd i) within the 128-wide k-block.
  - ACT: build diag(scales) from a constant identity via per-partition scale.
  - PE:  psum[i', o'] = qn[o, i'].T @ diag(s)[o, o']  -- one regular N=128
    matmul performs transpose AND scale: W.T tile = (q + z/s) * s = q*s + z.
  - evict psum -> SBUF fp16 stationary tile.
Main matmul accumulates y.T[o, t] over the 32 K-blocks in PSUM (N=512),
software-pipelined so dequant matmuls of the next row-block interleave 1:4
with main matmuls. x arrives as x.T in fp16 with rows permuted to match i'.
The host undoes nothing on the output path except transpose/concat + bias.
"""

import numpy as np

OUT, IN, BLOCK, TOKENS = 12288, 4096, 128, 2048
N_CORES = 8
OSH = OUT // N_CORES          # 1536 out rows per core
N_OT = OSH // 128             # 12 row-blocks of 128
N_B = IN // BLOCK             # 32 k-blocks of 128
N_NCH = TOKENS // 512         # 4 moving chunks of 512

_CACHED = {}


def _build_nc():
    import concourse.bacc as bacc
    import concourse.mybir as mybir
    import concourse.tile as tile
    from contextlib import ExitStack

    nc = bacc.Bacc("TRN2", target_bir_lowering=False)
    f16, f32, i32 = mybir.dt.float16, mybir.dt.float32, mybir.dt.int32

    pk_d = nc.dram_tensor("pk", [OSH, 2048], i32, kind="ExternalInput")
    xt_d = nc.dram_tensor("xt", [IN, TOKENS], f16, kind="ExternalInput")
    sv_d = nc.dram_tensor("sv", [128, N_OT * N_B], f32, kind="ExternalInput")
    zv_d = nc.dram_tensor("zv", [128, N_OT * N_B], f32, kind="ExternalInput")
    id_d = nc.dram_tensor("ident", [128, 128], f16, kind="ExternalInput")
    yt_d = nc.dram_tensor("yt", [OSH, TOKENS], f16, kind="ExternalOutput")

    RSH = mybir.AluOpType.logical_shift_right
    AND = mybir.AluOpType.bitwise_and
    ADD = mybir.AluOpType.add
    MULT = mybir.AluOpType.mult
    COPY = mybir.ActivationFunctionType.Copy

    with tile.TileContext(nc) as tc, ExitStack() as ctx:
        const = ctx.enter_context(tc.tile_pool(name="const", bufs=1))
        xpool = ctx.enter_context(tc.tile_pool(name="xpool", bufs=1))
        pkpool = ctx.enter_context(tc.tile_pool(name="pkpool", bufs=4))
        wtpool = ctx.enter_context(tc.tile_pool(name="wtpool", bufs=2))
        qpool = ctx.enter_context(tc.tile_pool(name="qpool", bufs=4))
        ypool = ctx.enter_context(tc.tile_pool(name="ypool", bufs=3))
        psw = ctx.enter_context(tc.tile_pool(name="psw", bufs=2, space="PSUM"))
        psy = ctx.enter_context(tc.tile_pool(name="psy", bufs=6, space="PSUM"))

        sv_sb = const.tile([128, N_OT * N_B], f32, name="sv_sb")
        zv_sb = const.tile([128, N_OT * N_B], f32, name="zv_sb")
        id_sb = const.tile([128, 128], f16, name="id_sb")
        # dependency-free warmup op so the ACT function-table load runs
        # during the NEFF preamble instead of before the first diag build
        warm = const.tile([128, 1], f32, name="warm")
        nc.vector.memset(warm[:], 0.0)
        nc.scalar.activation(warm[:], warm[:], COPY)
        # PE pre-warm: dep-free matmuls on memset tiles run during the
        # initial DMA shadow so the HAM clock gate is at 8/8 (2.4 GHz) by
        # the time the first real matmul issues.
        wm_sta = const.tile([128, 128], f16, name="wm_sta")
        wm_mov = const.tile([128, 512], f16, name="wm_mov")
        nc.vector.memset(wm_sta[:], 0.0)
        nc.vector.memset(wm_mov[:], 0.0)

        # resident x.T: one big SBUF tile, free index = b*2048 + t.
        # b=0..1 first so the first main matmuls can start early.
        xt_sb = xpool.tile([128, N_B * 2048], f16, name="xt_sb")

        def load_xt(b):
            nc.sync.dma_start(
                xt_sb[:, b * 2048:(b + 1) * 2048],
                xt_d[b * 128:(b + 1) * 128, :],
            )

        # packed half-row-block tiles (16 k-blocks each), keyed (ot, half)
        pk_tiles = {}

        def load_packed(ot):
            for h in range(2):
                t = pkpool.tile([128, 1024], i32, name="pk_sb", tag="pk_sb")
                nc.sync.dma_start(
                    t[:], pk_d[ot * 128:(ot + 1) * 128, h * 1024:(h + 1) * 1024])
                pk_tiles[(ot, h)] = t

        # DMA issue order is the first-matmul critical path: the tiny pk
        # head plus sv/zv heads and the identity go first, then xt b=0;
        # bulk loads follow.  ot0-h0 is split so a small head DMA lets the
        # first dequant chain start before the bulk lands.
        pk_head = {}
        t = pkpool.tile([128, 1024], i32, name="pk_sb", tag="pk_sb")
        nc.sync.dma_start(t[:, 0:256], pk_d[0:128, 0:256])
        pk_head[0] = t
        nc.sync.dma_start(sv_sb[:, 0:2 * N_B], sv_d[:, 0:2 * N_B])
        nc.sync.dma_start(zv_sb[:, 0:2 * N_B], zv_d[:, 0:2 * N_B])
        nc.sync.dma_start(id_sb[:], id_d[:, :])
        load_xt(0)
        nc.sync.dma_start(t[:, 256:1024], pk_d[0:128, 256:1024])
        pk_tiles[(0, 0)] = t
        t2 = pkpool.tile([128, 1024], i32, name="pk_sb", tag="pk_sb")
        nc.sync.dma_start(t2[:], pk_d[0:128, 1024:2048])
        pk_tiles[(0, 1)] = t2
        load_xt(1)
        # pk1 h0 split like pk0: a small head right after xt1 so ot1 can
        # join the phase-1 interleave by block 2; the bulk rides between
        # early xt blocks where the supply bubble is cheapest to absorb
        t3 = pkpool.tile([128, 1024], i32, name="pk_sb", tag="pk_sb")
        nc.sync.dma_start(t3[:, 0:256], pk_d[128:256, 0:256])
        load_xt(2)
        nc.sync.dma_start(t3[:, 256:1024], pk_d[128:256, 256:1024])
        pk_tiles[(1, 0)] = t3
        load_xt(3)
        t4 = pkpool.tile([128, 1024], i32, name="pk_sb", tag="pk_sb")
        nc.sync.dma_start(t4[:], pk_d[128:256, 1024:2048])
        pk_tiles[(1, 1)] = t4
        for b in range(4, N_B):
            load_xt(b)
            if b == 20:
                load_packed(2)
            elif b == 24:
                load_packed(3)
            elif b == 28:
                nc.sync.dma_start(sv_sb[:, 2 * N_B:], sv_d[:, 2 * N_B:])
                nc.sync.dma_start(zv_sb[:, 2 * N_B:], zv_d[:, 2 * N_B:])

        # pre-warm matmuls (see wm_sta/wm_mov above): ~3.8 us of dep-free PE
        # activity (first ~8 run at the cold 1.2 GHz clock) tripping the HAM
        # un-throttle before the first real matmul issues.  They share the
        # psy rotation so no extra PSUM banks are needed.
        for i in range(24):
            wp = psy.tile([128, 512], f32, name="py", tag="py")
            nc.tensor.matmul(wp[:], lhsT=wm_sta[:], rhs=wm_mov[:],
                             start=True, stop=True)

        def make_wt(ot):
            """Produce the [128 i', 32*128 o'] fp16 stationary tiles for ot.

            Returns per-b emit closures so callers can interleave them with
            main matmuls.
            """
            wt = wtpool.tile([128, N_B * 128], f16, name="wt_sb", tag="wt_sb")
            # batched nibble extracts per half-row-block (bitwise ops cannot
            # cast, so stay int32): layout per half = [hi 1024 | lo 1024].
            # Emission is split out so callers control where the extracts
            # land in engine program order (qraw slot recycling).
            qr3s = [None, None]
            self_q = [None, None]

            def emit_extract(h, part="all"):
                if part in ("all", "head"):
                    qraw = qpool.tile([128, 2048], i32, name="qraw",
                                      tag="qraw", bufs=3)
                    qr3s[h] = qraw[:].rearrange("p (h c) -> p h c", h=2)
                    self_q[h] = qraw
                else:
                    qraw = self_q[h]
                pk_sb = pk_tiles[(ot, h)]
                if ot <= 1 and h == 0:
                    # head (blocks 0..3) first: depends only on the small
                    # head DMA, so the dequant chain starts early; the rest
                    # is emitted separately so a late bulk DMA does not
                    # head-of-line block the DVE queue
                    if part in ("all", "head"):
                        nc.vector.tensor_scalar(qraw[:, 0:256], pk_sb[:, 0:256], 4, None, RSH)
                        nc.vector.tensor_scalar(qraw[:, 1024:1280], pk_sb[:, 0:256], 15, None, AND)
                    if part in ("all", "rest"):
                        nc.vector.tensor_scalar(qraw[:, 256:1024], pk_sb[:, 256:1024], 4, None, RSH)
                        nc.vector.tensor_scalar(qraw[:, 1280:2048], pk_sb[:, 256:1024], 15, None, AND)
                else:
                    nc.vector.tensor_scalar(qraw[:, 0:1024], pk_sb[:], 4, None, RSH)
                    nc.vector.tensor_scalar(qraw[:, 1024:2048], pk_sb[:], 15, None, AND)

            def emit_b(b):
                idx = ot * N_B + b
                qr3, lb = qr3s[b // 16], b % 16
                # convert + full dequant in one arith op: (q + z/s) * s =
                # q*s + z, per-partition scalars from zv/sv; input is the
                # two 64-wide chunks (hi, lo) of block b.  The PE matmul
                # against the constant identity then only transposes, so no
                # per-block diag build is needed (keeps Scalar off the
                # dequant critical path).
                qn = qpool.tile([128, 128], f16, name="qn", tag="qn")
                qn3 = qn[:].rearrange("p (h c) -> p h c", h=2)
                nc.vector.tensor_scalar(
                    qn3[:, :, :], qr3[:, :, lb * 64:(lb + 1) * 64],
                    zv_sb[:, idx:idx + 1], sv_sb[:, idx:idx + 1], ADD, MULT)
                pw = psw.tile([128, 128], f32, name="pw", tag="pw")
                nc.tensor.matmul(pw[:], lhsT=qn[:], rhs=id_sb[:],
                                 start=True, stop=True)
                nc.vector.tensor_copy(wt[:, b * 128:(b + 1) * 128], pw[:])

            return wt, emit_extract, emit_b

        def mm(pys_t, wt_t, b, nch, start, stop):
            nc.tensor.matmul(
                pys_t[:],
                lhsT=wt_t[:, b * 128:(b + 1) * 128],
                rhs=xt_sb[:, b * 2048 + nch * 512: b * 2048 + nch * 512 + 512],
                start=start, stop=stop)

        def evict(pys_t, ot, nch):
            # fp16 eviction halves the output DMA; the store is issued
            # from the Scalar HWDGE queue right after the copy on the
            # same engine (no cross-engine semaphore hop, and it keeps
            # the Sync queue free for input prefetches).
            y_sb = ypool.tile([128, 512], f16, name="y_sb", tag="y_sb")
            nc.scalar.copy(y_sb[:], pys_t[:])
            nc.scalar.dma_start(
                yt_d[ot * 128:(ot + 1) * 128, nch * 512:(nch + 1) * 512],
                y_sb[:])

        # --- phase 1: ot0 (4 chunks) + ot1 (chunks 0-1) share the xt
        # stream.  The xt DMA takes ~49 us; one row-block's matmuls are
        # only ~28 us, so a warm PE starves on ot0 alone.  Interleaving
        # 6 moving chunks per xt block (~1.4 us of PE work per 1.5 us of
        # stream) keeps the PE tracking the stream within the 6-bank psy
        # budget (psw keeps 2 banks so the dequant chain stays pipelined).
        # ot1's chunks 2-3 run as catch-up passes once xt is resident.
        wt0, ext0, emit0 = make_wt(0)
        ext0(0)
        ext0(1)
        wt1, ext1, emit1 = make_wt(1)
        pys0 = [psy.tile([128, 512], f32, name="py", tag="py")
                for _ in range(4)]
        pys1 = [psy.tile([128, 512], f32, name="py", tag="py")
                for _ in range(2)]
        def filler(pys_t):
            # dependency-free PE busywork: accumulate +0.0 (zero stationary)
            # into an open psum group -- numerically exact no-op.  Keeps the
            # HAM activity window from seeing a >3.4us idle (which would
            # re-throttle the PE to 1.2 GHz) while the xt stream is the
            # binding resource.
            nc.tensor.matmul(pys_t[:, 0:128], lhsT=wm_sta[:],
                             rhs=wm_mov[:, 0:128], start=False, stop=False)

        for b in range(N_B):
            # ot1 extracts staged so ot0's chain leads the DVE queue, the
            # late pk1 bulk DMAs never head-of-line block the DVE, and
            # qraw slots (3 bufs) recycle cleanly
            if b == 2:
                ext1(0, "head")
            elif b == 6:
                ext1(0, "rest")
            elif b == 18:
                ext1(1)
            emit0(b)
            for nch in range(4):
                mm(pys0[nch], wt0, b, nch, b == 0, b == N_B - 1)
            if 1 <= b <= 16:
                filler(pys0[0])
                filler(pys0[1])
            if b >= 4:
                emit1(b - 4)
                for nch in range(2):
                    mm(pys1[nch], wt1, b - 4, nch, b - 4 == 0, False)
        for b in range(N_B - 4, N_B):
            emit1(b)
            for nch in range(2):
                mm(pys1[nch], wt1, b, nch, False, b == N_B - 1)
        for nch in range(4):
            evict(pys0[nch], 0, nch)
        for nch in range(2):
            evict(pys1[nch], 1, nch)

        # ot1 chunks 2-3 catch-up passes, ot2's dequant interleaved in the
        # first one
        wt2, ext2, emit2 = make_wt(2)
        ext2(0)
        ext2(1)
        py12 = psy.tile([128, 512], f32, name="py", tag="py")
        for b in range(N_B):
            emit2(b)
            mm(py12, wt1, b, 2, b == 0, b == N_B - 1)
        evict(py12, 1, 2)
        py13 = psy.tile([128, 512], f32, name="py", tag="py")
        for b in range(N_B):
            mm(py13, wt1, b, 3, b == 0, b == N_B - 1)
        evict(py13, 1, 3)

        # --- phase 2: ot2..ot11, software-pipelined as before ---
        wt_cur, emit_cur = wt2, emit2
        for ot in range(2, N_OT):
            last = ot == N_OT - 1
            if ot + 2 < N_OT:
                load_packed(ot + 2)
            if ot + 1 < N_OT:
                wt_next, ext_next, emit_next = make_wt(ot + 1)
                ext_next(0)
                ext_next(1)
            else:
                wt_next, ext_next, emit_next = None, None, None

            pys = [psy.tile([128, 512], f32, name="py", tag="py")
                   for _ in range(N_NCH)]

            if not last:
                for b in range(N_B):
                    if emit_next is not None:
                        emit_next(b)
                    for nch in range(N_NCH):
                        mm(pys[nch], wt_cur, b, nch, b == 0, b == N_B - 1)
                for nch in range(N_NCH):
                    evict(pys[nch], ot, nch)
            else:
                # last row-block: nch-major so evicts/stores overlap the
                # trailing matmuls instead of serializing after them
                for nch in range(N_NCH):
                    for b in range(N_B):
                        mm(pys[nch], wt_cur, b, nch, b == 0, b == N_B - 1)
                    evict(pys[nch], ot, nch)
            wt_cur, emit_cur = wt_next, emit_next
            del ext_next

    nc.compile()
    return nc


def _host_prep(x, packed, scales, zeros):
    # i' permutation within each 128-block: evens (hi nibbles) then odds
    perm = np.empty(BLOCK, dtype=np.int64)
    perm[:64] = np.arange(64) * 2
    perm[64:] = np.arange(64) * 2 + 1
    full_perm = (np.arange(IN) // BLOCK) * BLOCK
    full_perm = full_perm + np.tile(perm, IN // BLOCK)

    xt = np.ascontiguousarray(x.T)[full_perm].astype(np.float16)

    pk2 = packed.reshape(OUT, IN // 2).astype(np.int32)
    s2 = scales.reshape(OUT, N_B).astype(np.float32)
    zos2 = (zeros.astype(np.float64) / scales.astype(np.float64))
    zos2 = zos2.reshape(OUT, N_B).astype(np.float32)

    ident = np.eye(128, dtype=np.float16)

    in_maps = []
    for c in range(N_CORES):
        rows = slice(c * OSH, (c + 1) * OSH)
        sv = np.ascontiguousarray(
            s2[rows].reshape(N_OT, 128, N_B).transpose(1, 0, 2).reshape(128, N_OT * N_B))
        zv = np.ascontiguousarray(
            zos2[rows].reshape(N_OT, 128, N_B).transpose(1, 0, 2).reshape(128, N_OT * N_B))
        in_maps.append({
            "pk": np.ascontiguousarray(pk2[rows]),
            "xt": xt,
            "sv": sv,
            "zv": zv,
            "ident": ident,
        })
    return in_maps


def kernel(x, packed, scales, zeros, bias):
    from concourse.bass_utils import run_bass_kernel_spmd

    x = np.asarray(x, dtype=np.float32)
    packed = np.asarray(packed, dtype=np.int32)
    scales = np.asarray(scales, dtype=np.float32)
    zeros = np.asarray(zeros, dtype=np.float32)
    bias = np.asarray(bias, dtype=np.float32)

    if "nc" not in _CACHED:
        _CACHED["nc"] = _build_nc()
    nc = _CACHED["nc"]

    in_maps = _host_prep(x, packed, scales, zeros)
    res = run_bass_kernel_spmd(nc, in_maps, core_ids=list(range(N_CORES)))
    yt = np.concatenate(
        [np.asarray(res.results[c]["yt"]) for c in range(N_CORES)], axis=0)
    y = yt.astype(np.float32).T + bias.astype(np.float32)[None, :]
    return np.ascontiguousarray(y)

